# revision 5
# baseline (speedup 1.0000x reference)
"""TRN2 Bass kernel for nn_Basicblock (binarized CNN basic block).

Strategy: data-parallel over batch (4 images per core x 8 cores).
Binary convs run as fp8 DoubleRow matmuls (K=256) with fp32 PSUM.
Training-mode BN uses global batch stats via tiny on-device AllReduces.

v3 engine assignment (per-element passes):
- ACT: sign1, copy1(+row-sum accum), preluB, copy2(+accum), preluD
- DVE: sum-of-squares via tensor_tensor_reduce (scaled 1/64, f16 dump),
       scalar_tensor_tensor BN-scale+residual (in-place), small coef math
- GPSIMD: conv2-input sign as (v >= thr) - 0.5 (scale folded into BN2
  consts on host), final bias add
- x read from HBM once, resident in SBUF; p (f16) reuses x's pool slots.
"""
import os
import sys

sys.path.insert(0, "/opt/trn_rl_repo")
os.environ.setdefault("MYCRO_LOCAL_CACHE", "1")

import numpy as np

import concourse.bass as bass
import concourse.mybir as mybir
import concourse.tile as tile
from concourse import bacc, bass_utils
from contextlib import ExitStack

F32 = mybir.dt.float32
F16 = mybir.dt.float16
F8 = mybir.dt.float8e4
AF = mybir.ActivationFunctionType
ALU = mybir.AluOpType

NCORES = 8
P = 128
IMGS = 4            # images per core
H = W = 56
HP = 58             # padded spatial
PIXI = H * W        # 3136
PIXC = IMGS * PIXI  # 12544 pixels per channel-chunk per core
NG = 7              # 8-row groups per image
NMM = 8 * W         # 448 matmul free dim
NMT = 7             # megatiles per oc chunk (4 groups each)
MEGA = 4 * NMM      # 1792 pixels per megatile
QB = 1568           # phase-D block (half image)
BB = 3136           # phase-B block (full image)
SQS = 64.0          # sum-of-squares dump scale (keeps f16 exact)
NTOT = float(32 * PIXI)
EPS = 1e-5
NCON = 11
GP_SIGN2 = True
GP_FINAL = True

_nc_cache = {}


def _build():
    nc = bacc.Bacc("TRN2", target_bir_lowering=False, debug=False,
                   enable_asserts=False, num_devices=NCORES)
    DR = mybir.MatmulPerfMode.DoubleRow
    x_d = nc.dram_tensor("x", [IMGS, 256, H, W], F32, kind="ExternalInput").ap()
    w1_d = nc.dram_tensor("w1", [P, 18, 2, P], F8, kind="ExternalInput").ap()
    w2_d = nc.dram_tensor("w2", [P, 2, 2, P], F8, kind="ExternalInput").ap()
    cst_d = nc.dram_tensor("consts", [P, 2, NCON], F32, kind="ExternalInput").ap()
    out_d = nc.dram_tensor("out", [IMGS, 256, H, W], F32, kind="ExternalOutput").ap()

    def x_flat(img, cc):
        return x_d[img, cc * P:(cc + 1) * P, :, :].rearrange("c h w -> c (h w)")

    def out_flat(img, oc):
        return out_d[img, oc * P:(oc + 1) * P, :, :].rearrange("c h w -> c (h w)")

    with tile.TileContext(nc) as tc, ExitStack() as ctx:
        kp = ctx.enter_context(tc.tile_pool(name="kp", bufs=1))
        smp = ctx.enter_context(tc.tile_pool(name="smp", bufs=24))
        xp = ctx.enter_context(tc.tile_pool(name="xp", bufs=2))
        yy = ctx.enter_context(tc.tile_pool(name="yy", bufs=2))
        big8 = ctx.enter_context(tc.tile_pool(name="big8", bufs=1))
        qp = ctx.enter_context(tc.tile_pool(name="qp", bufs=3))
        psp = ctx.enter_context(tc.tile_pool(name="psp", bufs=2, space="PSUM"))
        drp = ctx.enter_context(tc.tile_pool(name="drp", bufs=1, space="DRAM"))

        cst = kp.tile([P, 2, NCON], F32, name="cst")
        nc.sync.dma_start(cst[:], cst_d)
        w1s = kp.tile([P, 18, 2, P], F8, name="w1s")
        nc.sync.dma_start(w1s[:], w1_d)
        w2s = kp.tile([P, 2, 2, P], F8, name="w2s")
        nc.sync.dma_start(w2s[:], w2_d)

        # x resident; later reused (rotated) for p (f16)
        x_t = [xp.tile([P, PIXC], F32, name=f"x{c}", tag="xc") for c in (0, 1)]
        y1_t = [yy.tile([P, PIXC], F16, name=f"y1_{o}", tag="y") for o in (0, 1)]
        xpad = big8.tile([P, 2, IMGS, HP, HP], F8, name="xpad", tag="b8")
        bnst1 = [kp.tile([P, 4 * NMT, 6], F32, name=f"bnst1_{o}") for o in (0, 1)]
        bnst2 = [kp.tile([P, 4 * NMT, 6], F32, name=f"bnst2_{o}") for o in (0, 1)]
        a1p = [kp.tile([P, 1], F32, name=f"a1p{o}") for o in (0, 1)]
        c1b = [kp.tile([P, 1], F32, name=f"c1b{o}") for o in (0, 1)]
        th2 = [kp.tile([P, 1], F32, name=f"th2{o}") for o in (0, 1)]
        a2p = [kp.tile([P, 1], F32, name=f"a2p{o}") for o in (0, 1)]
        c2b = [kp.tile([P, 1], F32, name=f"c2b{o}") for o in (0, 1)]

        # ---------------- phase A: pad borders, sign(x+b1) -> xpad (fp8)
        nc.gpsimd.memset(xpad[:, :, :, 0, :], 0.0)
        nc.gpsimd.memset(xpad[:, :, :, HP - 1, :], 0.0)
        nc.gpsimd.memset(xpad[:, :, :, :, 0], 0.0)
        nc.gpsimd.memset(xpad[:, :, :, :, HP - 1], 0.0)
        for img in range(IMGS):
            for c in (0, 1):
                nc.sync.dma_start(
                    x_t[c][:, img * PIXI:(img + 1) * PIXI], x_flat(img, c))
                nc.scalar.activation(
                    xpad[:, c, img, 1:57, 1:57],
                    x_t[c][:, img * PIXI:(img + 1) * PIXI]
                    .rearrange("c (h w) -> c h w", w=W),
                    AF.Sign, bias=cst[:, c, 0:1])

        # ---------------- conv1: 3x3, DoubleRow K=256, PSUM megatiles
        def conv1_mega(oc, m):
            ps = psp.tile([P, 4, 512], F32, tag="mm", name="ps1")
            for sub in range(4):
                G = m * 4 + sub
                img, g = divmod(G, NG)
                for kidx in range(9):
                    dh, dw = divmod(kidx, 3)
                    nc.tensor.matmul(
                        ps[:, sub, 0:NMM],
                        w1s[:, oc * 9 + kidx, :, :],
                        xpad[:, :, img, g * 8 + dh:g * 8 + 8 + dh, dw:dw + W],
                        start=(kidx == 0), stop=(kidx == 8),
                        perf_mode=DR)
            sl = slice(m * MEGA, (m + 1) * MEGA)
            nc.scalar.activation(y1_t[oc][:, sl], ps[:, :, 0:NMM], AF.Copy)
            for sub in range(4):
                t = m * 4 + sub
                nc.vector.bn_stats(bnst1[oc][:, t, :],
                                   y1_t[oc][:, t * NMM:(t + 1) * NMM])

        def emit_stats(bnst, tag):
            agg = kp.tile([P, 2], F32, name=f"agg_{tag}")
            nc.vector.bn_aggr(agg[:], bnst[:].rearrange("p a b -> p (a b)"))
            pk = kp.tile([P, 2], F32, name=f"pk_{tag}")
            nc.vector.tensor_scalar_mul(pk[:, 0:1], agg[:, 0:1], float(PIXC))
            msq = smp.tile([P, 1], F32, tag="sm", name="sm")
            nc.vector.tensor_tensor(msq[:], agg[:, 0:1], agg[:, 0:1], ALU.mult)
            t2 = smp.tile([P, 1], F32, tag="sm", name="sm")
            nc.vector.tensor_tensor(t2[:], agg[:, 1:2], msq[:], ALU.add)
            nc.vector.tensor_scalar_mul(pk[:, 1:2], t2[:], float(PIXC))
            cin = drp.tile([P, 2], F32, name=f"cin_{tag}")
            cout = drp.tile([P, 2], F32, name=f"cout_{tag}", addr_space="Shared")
            nc.sync.dma_start(cin[:], pk[:])
            nc.gpsimd.collective_compute(
                "AllReduce", ALU.add, replica_groups=[list(range(NCORES))],
                ins=[cin.opt()], outs=[cout.opt()])
            gsb = kp.tile([P, 2], F32, name=f"gst_{tag}")
            nc.sync.dma_start(gsb[:], cout[:])
            return gsb

        def coef_math(gsb, oc, a_t, c_t, j_s2, j_gs, j_cb, thr_t=None, j_t2=None):
            # a = gs / sqrt(s^2*var_raw + eps); c = cb - a*mean_raw
            m = smp.tile([P, 1], F32, tag="sm", name="sm")
            nc.vector.tensor_scalar_mul(m[:], gsb[:, 0:1], 1.0 / NTOT)
            e2 = smp.tile([P, 1], F32, tag="sm", name="sm")
            nc.vector.tensor_scalar_mul(e2[:], gsb[:, 1:2], 1.0 / NTOT)
            msq = smp.tile([P, 1], F32, tag="sm", name="sm")
            nc.vector.tensor_tensor(msq[:], m[:], m[:], ALU.mult)
            vr = smp.tile([P, 1], F32, tag="sm", name="sm")
            nc.vector.tensor_tensor(vr[:], e2[:], msq[:], ALU.subtract)
            ve = smp.tile([P, 1], F32, tag="sm", name="sm")
            nc.vector.tensor_scalar(
                out=ve[:], in0=vr[:], scalar1=cst[:, oc, j_s2:j_s2 + 1],
                scalar2=EPS, op0=ALU.mult, op1=ALU.add)
            sd = smp.tile([P, 1], F32, tag="sm", name="sm")
            nc.scalar.activation(sd[:], ve[:], AF.Sqrt)
            inv = smp.tile([P, 1], F32, tag="sm", name="sm")
            nc.vector.reciprocal(inv[:], sd[:])
            nc.vector.tensor_scalar_mul(a_t[:], inv[:], cst[:, oc, j_gs:j_gs + 1])
            am = smp.tile([P, 1], F32, tag="sm", name="sm")
            nc.vector.tensor_tensor(am[:], a_t[:], m[:], ALU.mult)
            nc.vector.tensor_tensor(c_t[:], cst[:, oc, j_cb:j_cb + 1], am[:],
                                    ALU.subtract)
            if thr_t is not None:
                if GP_SIGN2:
                    # gpsimd sign is (v >= thr) - 0.5 with thr = -(c + t2)
                    tt = smp.tile([P, 1], F32, tag="sm", name="sm")
                    nc.vector.tensor_tensor(tt[:], c_t[:],
                                            cst[:, oc, j_t2:j_t2 + 1], ALU.add)
                    nc.vector.tensor_scalar_mul(thr_t[:], tt[:], -1.0)
                else:
                    nc.vector.tensor_tensor(thr_t[:], c_t[:],
                                            cst[:, oc, j_t2:j_t2 + 1], ALU.add)

        # conv2 input (fp8 +-0.5) + p tiles (reuse x slots)
        xb2 = big8.tile([P, 2, PIXC], F8, name="xb2", tag="b8")
        p_t = [xp.tile([P, PIXC], F16, name=f"p{c}", tag="xc") for c in (0, 1)]
        y2_t = [yy.tile([P, PIXC], F16, name=f"y2_{o}", tag="y") for o in (0, 1)]

        def phaseB_block(oc, b):
            sl = slice(b * BB, (b + 1) * BB)
            # v = a1*y1 + x  (in-place over y1, f16)
            nc.vector.scalar_tensor_tensor(
                out=y1_t[oc][:, sl], in0=y1_t[oc][:, sl], scalar=a1p[oc][:],
                in1=x_t[oc][:, sl], op0=ALU.mult, op1=ALU.add)
            # p = Prelu(v + c1)
            nc.scalar.activation(p_t[oc][:, sl], y1_t[oc][:, sl], AF.Prelu,
                                 bias=c1b[oc][:], alpha=cst[:, oc, 4:5])
            # conv2 input: sign(p + b') == sign(v + t) == (v >= -t) - 0.5
            if GP_SIGN2:
                nc.gpsimd.tensor_scalar(
                    out=xb2[:, oc, sl], in0=y1_t[oc][:, sl], scalar1=th2[oc][:],
                    scalar2=0.5, op0=ALU.is_ge, op1=ALU.subtract)
            else:
                nc.scalar.activation(xb2[:, oc, sl], y1_t[oc][:, sl], AF.Sign,
                                     bias=th2[oc][:])

        # ---------------- conv2 (1x1, DoubleRow K=256)
        def conv2_mega(oc, m):
            ps = psp.tile([P, 4, 512], F32, tag="mm", name="ps2")
            for sub in range(4):
                t = m * 4 + sub
                nc.tensor.matmul(ps[:, sub, 0:NMM], w2s[:, oc, :, :],
                                 xb2[:, :, t * NMM:(t + 1) * NMM],
                                 start=True, stop=True, perf_mode=DR)
            sl = slice(m * MEGA, (m + 1) * MEGA)
            nc.scalar.activation(y2_t[oc][:, sl], ps[:, :, 0:NMM], AF.Copy)
            for sub in range(4):
                t = m * 4 + sub
                nc.vector.bn_stats(bnst2[oc][:, t, :],
                                   y2_t[oc][:, t * NMM:(t + 1) * NMM])

        def phaseD_block(oc, b):
            sl = slice(b * QB, (b + 1) * QB)
            # v2 = a2*y2 + p  (in-place over y2, all f16)
            nc.vector.scalar_tensor_tensor(
                out=y2_t[oc][:, sl], in0=y2_t[oc][:, sl], scalar=a2p[oc][:],
                in1=p_t[oc][:, sl], op0=ALU.mult, op1=ALU.add)
            q = qp.tile([P, QB], F32, tag="q", name="q")
            nc.scalar.activation(q[:], y2_t[oc][:, sl], AF.Prelu,
                                 bias=c2b[oc][:], alpha=cst[:, oc, 9:10])
            # out = q + b2_3
            eng = nc.gpsimd if GP_FINAL else nc.vector
            eng.tensor_scalar(out=q[:], in0=q[:],
                              scalar1=cst[:, oc, 10:11], scalar2=0.0,
                              op0=ALU.add, op1=ALU.add)
            img, half = divmod(b, 2)
            nc.sync.dma_start(
                out_flat(img, oc)[:, half * QB:(half + 1) * QB], q[:])

        # ================= schedule =================
        for m in range(NMT):
            conv1_mega(0, m)
        gsb10 = emit_stats(bnst1[0], "10")
        for m in range(NMT):
            conv1_mega(1, m)
        gsb11 = emit_stats(bnst1[1], "11")
        coef_math(gsb10, 0, a1p[0], c1b[0], 2, 1, 3, th2[0], 5)
        # phase B oc0 fills the AllReduce-1(1) latency window
        for b in range(4):
            phaseB_block(0, b)
        coef_math(gsb11, 1, a1p[1], c1b[1], 2, 1, 3, th2[1], 5)
        for b in range(4):
            phaseB_block(1, b)

        for m in range(NMT):
            conv2_mega(0, m)
        gsb20 = emit_stats(bnst2[0], "20")
        for m in range(NMT):
            conv2_mega(1, m)
        gsb21 = emit_stats(bnst2[1], "21")
        coef_math(gsb20, 0, a2p[0], c2b[0], 7, 6, 8)
        for b in range(8):
            phaseD_block(0, b)
        coef_math(gsb21, 1, a2p[1], c2b[1], 7, 6, 8)
        for b in range(8):
            phaseD_block(1, b)

    nc.compile()
    return nc


def _get_nc():
    if "nc" not in _nc_cache:
        _nc_cache["nc"] = _build()
    return _nc_cache["nc"]


def _prep_inputs(inputs):
    f8np = mybir.dt.np(F8)
    x = np.ascontiguousarray(np.asarray(inputs["x"], np.float32))
    w3 = np.asarray(inputs["w3x3"], np.float32)
    wr = np.asarray(inputs["wres"], np.float32)
    s1 = np.abs(w3).mean(axis=(1, 2, 3))
    # with GP sign, conv2 activations are +-0.5 -> fold 2x into its scale
    s2 = np.abs(wr).mean(axis=(1, 2, 3))
    if GP_SIGN2:
        s2 = 2.0 * s2
    w1h = (np.sign(w3).reshape(2, P, 2, P, 3, 3).transpose(3, 0, 4, 5, 2, 1)
           .reshape(P, 18, 2, P)).astype(f8np)
    w2h = (np.sign(wr)[:, :, 0, 0].reshape(2, P, 2, P).transpose(3, 0, 2, 1)
           .reshape(P, 2, 2, P)).astype(f8np)

    def col(v):
        return np.asarray(v, np.float32).reshape(2, P).T

    g1 = np.asarray(inputs["bn1_g"], np.float32)
    be1 = np.asarray(inputs["bn1_b"], np.float32)
    g2 = np.asarray(inputs["bn2_g"], np.float32)
    be2 = np.asarray(inputs["bn2_b"], np.float32)
    b1_1, b1_2, b1_3 = (np.asarray(inputs[k], np.float32)
                        for k in ("b1_1", "b1_2", "b1_3"))
    b2_1, b2_2, b2_3 = (np.asarray(inputs[k], np.float32)
                        for k in ("b2_1", "b2_2", "b2_3"))
    pa1 = np.asarray(inputs["prelu1_a"], np.float32)
    pa2 = np.asarray(inputs["prelu2_a"], np.float32)
    # sign2 threshold offset: sign(Prelu(z)+b') == sign(z + t2), b'=b1_3+b2_1
    bp = b1_3 + b2_1
    t2 = np.where(bp >= 0, bp / pa1, bp).astype(np.float32)
    cols = [b1_1, g1 * s1, s1 * s1, be1 + b1_2, pa1, t2,
            g2 * s2, s2 * s2, be2 + b1_3 + b2_2, pa2, b2_3]
    csth = np.stack([col(v) for v in cols], axis=2).astype(np.float32)
    csth = np.ascontiguousarray(csth)

    in_maps = []
    for c in range(NCORES):
        in_maps.append({
            "x": np.ascontiguousarray(x[c * IMGS:(c + 1) * IMGS]),
            "w1": w1h, "w2": w2h, "consts": csth,
        })
    return in_maps


def _run(in_maps, trace=False):
    nc = _get_nc()
    return bass_utils.run_bass_kernel_spmd(
        nc, in_maps, core_ids=list(range(NCORES)), trace=trace)


def kernel(**inputs):
    in_maps = _prep_inputs(inputs)
    res = _run(in_maps)
    out = np.concatenate([res.results[c]["out"] for c in range(NCORES)], axis=0)
    return out.astype(np.float32)


# revision 7
# speedup vs baseline: 3.0416x; 3.0416x over previous
"""TRN2 Bass kernel for nn_Basicblock (binarized CNN basic block).

Strategy: data-parallel over batch (4 images per core x 8 cores).
Binary convs run as fp8 DoubleRow matmuls (K=256) with fp32 PSUM.
Training-mode BN uses global batch stats via tiny on-device AllReduces.

v3 engine assignment (per-element passes):
- ACT: sign1, copy1(+row-sum accum), preluB, copy2(+accum), preluD
- DVE: sum-of-squares via tensor_tensor_reduce (scaled 1/64, f16 dump),
       scalar_tensor_tensor BN-scale+residual (in-place), small coef math
- GPSIMD: conv2-input sign as (v >= thr) - 0.5 (scale folded into BN2
  consts on host), final bias add
- x read from HBM once, resident in SBUF; p (f16) reuses x's pool slots.
"""
import os
import sys

sys.path.insert(0, "/opt/trn_rl_repo")
os.environ.setdefault("MYCRO_LOCAL_CACHE", "1")

import numpy as np

import concourse.bass as bass
import concourse.mybir as mybir
import concourse.tile as tile
from concourse import bacc, bass_utils
from contextlib import ExitStack

F32 = mybir.dt.float32
F16 = mybir.dt.float16
F8 = mybir.dt.float8e4
AF = mybir.ActivationFunctionType
ALU = mybir.AluOpType

NCORES = 8
P = 128
IMGS = 4            # images per core
H = W = 56
HP = 58             # padded spatial
PIXI = H * W        # 3136
PIXC = IMGS * PIXI  # 12544 pixels per channel-chunk per core
NG = 7              # 8-row groups per image
NMM = 8 * W         # 448 matmul free dim
NMT = 7             # megatiles per oc chunk (4 groups each)
MEGA = 4 * NMM      # 1792 pixels per megatile
QB = 1568           # phase-D block (half image)
BB = 3136           # phase-B block (full image)
SQS = 64.0          # sum-of-squares dump scale (keeps f16 exact)
NTOT = float(32 * PIXI)
EPS = 1e-5
NCON = 11

_nc_cache = {}


def _build():
    nc = bacc.Bacc("TRN2", target_bir_lowering=False, debug=False,
                   enable_asserts=False, num_devices=NCORES)
    DR = mybir.MatmulPerfMode.DoubleRow
    x_d = nc.dram_tensor("x", [IMGS, 256, H, W], F32, kind="ExternalInput").ap()
    w1_d = nc.dram_tensor("w1", [P, 18, 2, P], F8, kind="ExternalInput").ap()
    w2_d = nc.dram_tensor("w2", [P, 2, 2, P], F8, kind="ExternalInput").ap()
    cst_d = nc.dram_tensor("consts", [P, 2, NCON], F32, kind="ExternalInput").ap()
    out_d = nc.dram_tensor("out", [IMGS, 256, H, W], F32, kind="ExternalOutput").ap()

    def x_flat(img, cc):
        return x_d[img, cc * P:(cc + 1) * P, :, :].rearrange("c h w -> c (h w)")

    def out_flat(img, oc):
        return out_d[img, oc * P:(oc + 1) * P, :, :].rearrange("c h w -> c (h w)")

    with tile.TileContext(nc) as tc, ExitStack() as ctx:
        kp = ctx.enter_context(tc.tile_pool(name="kp", bufs=1))
        smp = ctx.enter_context(tc.tile_pool(name="smp", bufs=24))
        xp = ctx.enter_context(tc.tile_pool(name="xp", bufs=2))
        yy = ctx.enter_context(tc.tile_pool(name="yy", bufs=2))
        big8 = ctx.enter_context(tc.tile_pool(name="big8", bufs=1))
        qp = ctx.enter_context(tc.tile_pool(name="qp", bufs=3))
        vp = ctx.enter_context(tc.tile_pool(name="vp", bufs=3))
        psp = ctx.enter_context(tc.tile_pool(name="psp", bufs=2, space="PSUM"))
        drp = ctx.enter_context(tc.tile_pool(name="drp", bufs=1, space="DRAM"))

        cst = kp.tile([P, 2, NCON], F32, name="cst")
        nc.sync.dma_start(cst[:], cst_d)
        w1s = kp.tile([P, 18, 2, P], F8, name="w1s")
        nc.sync.dma_start(w1s[:], w1_d)
        w2s = kp.tile([P, 2, 2, P], F8, name="w2s")
        nc.sync.dma_start(w2s[:], w2_d)

        # x resident as f16 (gpsimd cast-DMA); slots later reused for p
        x_t = [xp.tile([P, PIXC], F16, name=f"x{c}", tag="xc") for c in (0, 1)]
        y1_t = [yy.tile([P, PIXC], F16, name=f"y1_{o}", tag="y") for o in (0, 1)]
        xpad = big8.tile([P, 2, IMGS, HP, HP], F8, name="xpad", tag="b8")
        bnst1 = [kp.tile([P, 4 * NMT, 6], F32, name=f"bnst1_{o}") for o in (0, 1)]
        bnst2 = [kp.tile([P, 4 * NMT, 6], F32, name=f"bnst2_{o}") for o in (0, 1)]
        a1p = [kp.tile([P, 1], F32, name=f"a1p{o}") for o in (0, 1)]
        c1b = [kp.tile([P, 1], F32, name=f"c1b{o}") for o in (0, 1)]
        th2 = [kp.tile([P, 1], F32, name=f"th2{o}") for o in (0, 1)]
        a2p = [kp.tile([P, 1], F32, name=f"a2p{o}") for o in (0, 1)]
        c2b = [kp.tile([P, 1], F32, name=f"c2b{o}") for o in (0, 1)]

        # ---------------- phase A: pad borders, sign(x+b1) -> xpad (fp8)
        nc.gpsimd.memset(xpad[:, :, :, 0, :], 0.0)
        nc.gpsimd.memset(xpad[:, :, :, HP - 1, :], 0.0)
        nc.gpsimd.memset(xpad[:, :, :, :, 0], 0.0)
        nc.gpsimd.memset(xpad[:, :, :, :, HP - 1], 0.0)
        for img in range(IMGS):
            for c in (0, 1):
                nc.gpsimd.dma_start(
                    x_t[c][:, img * PIXI:(img + 1) * PIXI], x_flat(img, c))
                nc.scalar.activation(
                    xpad[:, c, img, 1:57, 1:57],
                    x_t[c][:, img * PIXI:(img + 1) * PIXI]
                    .rearrange("c (h w) -> c h w", w=W),
                    AF.Sign, bias=cst[:, c, 0:1])

        # ---------------- conv1: 3x3, DoubleRow K=256, PSUM megatiles
        def conv1_mega(oc, m):
            ps = psp.tile([P, 4, 512], F32, tag="mm", name="ps1")
            for sub in range(4):
                G = m * 4 + sub
                img, g = divmod(G, NG)
                for kidx in range(9):
                    dh, dw = divmod(kidx, 3)
                    nc.tensor.matmul(
                        ps[:, sub, 0:NMM],
                        w1s[:, oc * 9 + kidx, :, :],
                        xpad[:, :, img, g * 8 + dh:g * 8 + 8 + dh, dw:dw + W],
                        start=(kidx == 0), stop=(kidx == 8),
                        perf_mode=DR)
            sl = slice(m * MEGA, (m + 1) * MEGA)
            nc.scalar.activation(y1_t[oc][:, sl], ps[:, :, 0:NMM], AF.Copy)
            for sub in range(4):
                t = m * 4 + sub
                nc.vector.bn_stats(bnst1[oc][:, t, :],
                                   y1_t[oc][:, t * NMM:(t + 1) * NMM])

        def emit_stats(bnst, tag):
            agg = kp.tile([P, 2], F32, name=f"agg_{tag}")
            nc.vector.bn_aggr(agg[:], bnst[:].rearrange("p a b -> p (a b)"))
            pk = kp.tile([P, 2], F32, name=f"pk_{tag}")
            nc.vector.tensor_scalar_mul(pk[:, 0:1], agg[:, 0:1], float(PIXC))
            msq = smp.tile([P, 1], F32, tag="sm", name="sm")
            nc.vector.tensor_tensor(msq[:], agg[:, 0:1], agg[:, 0:1], ALU.mult)
            t2 = smp.tile([P, 1], F32, tag="sm", name="sm")
            nc.vector.tensor_tensor(t2[:], agg[:, 1:2], msq[:], ALU.add)
            nc.vector.tensor_scalar_mul(pk[:, 1:2], t2[:], float(PIXC))
            cin = drp.tile([P, 2], F32, name=f"cin_{tag}")
            cout = drp.tile([P, 2], F32, name=f"cout_{tag}", addr_space="Shared")
            nc.sync.dma_start(cin[:], pk[:])
            nc.gpsimd.collective_compute(
                "AllReduce", ALU.add, replica_groups=[list(range(NCORES))],
                ins=[cin.opt()], outs=[cout.opt()])
            gsb = kp.tile([P, 2], F32, name=f"gst_{tag}")
            nc.sync.dma_start(gsb[:], cout[:])
            return gsb

        def coef_math(gsb, oc, a_t, c_t, j_s2, j_gs, j_cb, thr_t=None, j_t2=None):
            # a = gs / sqrt(s^2*var_raw + eps); c = cb - a*mean_raw
            m = smp.tile([P, 1], F32, tag="sm", name="sm")
            nc.vector.tensor_scalar_mul(m[:], gsb[:, 0:1], 1.0 / NTOT)
            e2 = smp.tile([P, 1], F32, tag="sm", name="sm")
            nc.vector.tensor_scalar_mul(e2[:], gsb[:, 1:2], 1.0 / NTOT)
            msq = smp.tile([P, 1], F32, tag="sm", name="sm")
            nc.vector.tensor_tensor(msq[:], m[:], m[:], ALU.mult)
            vr = smp.tile([P, 1], F32, tag="sm", name="sm")
            nc.vector.tensor_tensor(vr[:], e2[:], msq[:], ALU.subtract)
            ve = smp.tile([P, 1], F32, tag="sm", name="sm")
            nc.vector.tensor_scalar(
                out=ve[:], in0=vr[:], scalar1=cst[:, oc, j_s2:j_s2 + 1],
                scalar2=EPS, op0=ALU.mult, op1=ALU.add)
            sd = smp.tile([P, 1], F32, tag="sm", name="sm")
            nc.scalar.activation(sd[:], ve[:], AF.Sqrt)
            inv = smp.tile([P, 1], F32, tag="sm", name="sm")
            nc.vector.reciprocal(inv[:], sd[:])
            nc.vector.tensor_scalar_mul(a_t[:], inv[:], cst[:, oc, j_gs:j_gs + 1])
            am = smp.tile([P, 1], F32, tag="sm", name="sm")
            nc.vector.tensor_tensor(am[:], a_t[:], m[:], ALU.mult)
            nc.vector.tensor_tensor(c_t[:], cst[:, oc, j_cb:j_cb + 1], am[:],
                                    ALU.subtract)
            if thr_t is not None:
                # DVE sign is (v >= thr) - 0.5 with thr = -(c + t2)
                tt = smp.tile([P, 1], F32, tag="sm", name="sm")
                nc.vector.tensor_tensor(tt[:], c_t[:],
                                        cst[:, oc, j_t2:j_t2 + 1], ALU.add)
                nc.vector.tensor_scalar_mul(thr_t[:], tt[:], -1.0)

        # conv2 input (fp8 +-0.5) + p tiles (reuse x slots)
        xb2 = big8.tile([P, 2, PIXC], F8, name="xb2", tag="b8")
        p_t = [xp.tile([P, PIXC], F16, name=f"p{c}", tag="xc") for c in (0, 1)]
        y2_t = [yy.tile([P, PIXC], F16, name=f"y2_{o}", tag="y") for o in (0, 1)]

        def phaseB_block(oc, b):
            sl = slice(b * BB, (b + 1) * BB)
            v = vp.tile([P, BB], F16, tag="v", name="v", bufs=4)
            # v = a1*y1 + x  (all f16)
            nc.vector.scalar_tensor_tensor(
                out=v[:], in0=y1_t[oc][:, sl], scalar=a1p[oc][:],
                in1=x_t[oc][:, sl], op0=ALU.mult, op1=ALU.add)
            # p = Prelu(v + c1)
            nc.scalar.activation(p_t[oc][:, sl], v[:], AF.Prelu,
                                 bias=c1b[oc][:], alpha=cst[:, oc, 4:5])
            # conv2 input: sign(p + b') == sign(v + t) == (v >= -t) - 0.5
            nc.vector.tensor_scalar(
                out=xb2[:, oc, sl], in0=v[:], scalar1=th2[oc][:],
                scalar2=0.5, op0=ALU.is_ge, op1=ALU.subtract)

        # ---------------- conv2 (1x1, DoubleRow K=256)
        def conv2_mega(oc, m):
            ps = psp.tile([P, 4, 512], F32, tag="mm", name="ps2")
            for sub in range(4):
                t = m * 4 + sub
                nc.tensor.matmul(ps[:, sub, 0:NMM], w2s[:, oc, :, :],
                                 xb2[:, :, t * NMM:(t + 1) * NMM],
                                 start=True, stop=True, perf_mode=DR)
            sl = slice(m * MEGA, (m + 1) * MEGA)
            nc.scalar.activation(y2_t[oc][:, sl], ps[:, :, 0:NMM], AF.Copy)
            for sub in range(4):
                t = m * 4 + sub
                nc.vector.bn_stats(bnst2[oc][:, t, :],
                                   y2_t[oc][:, t * NMM:(t + 1) * NMM])

        def phaseD_block(oc, b):
            sl = slice(b * QB, (b + 1) * QB)
            v2 = vp.tile([P, QB], F16, tag="v2", name="v2")
            # v2 = a2*y2 + p  (all f16)
            nc.vector.scalar_tensor_tensor(
                out=v2[:], in0=y2_t[oc][:, sl], scalar=a2p[oc][:],
                in1=p_t[oc][:, sl], op0=ALU.mult, op1=ALU.add)
            q = qp.tile([P, QB], F32, tag="q", name="q")
            nc.scalar.activation(q[:], v2[:], AF.Prelu,
                                 bias=c2b[oc][:], alpha=cst[:, oc, 9:10])
            # out = q + b2_3
            nc.vector.tensor_scalar(out=q[:], in0=q[:],
                                    scalar1=cst[:, oc, 10:11], scalar2=0.0,
                                    op0=ALU.add, op1=ALU.add)
            img, half = divmod(b, 2)
            nc.sync.dma_start(
                out_flat(img, oc)[:, half * QB:(half + 1) * QB], q[:])

        # ================= schedule =================
        for m in range(NMT):
            conv1_mega(0, m)
        gsb10 = emit_stats(bnst1[0], "10")
        for m in range(NMT):
            conv1_mega(1, m)
        gsb11 = emit_stats(bnst1[1], "11")
        coef_math(gsb10, 0, a1p[0], c1b[0], 2, 1, 3, th2[0], 5)
        # phase B oc0 fills the AllReduce-1(1) latency window
        for b in range(4):
            phaseB_block(0, b)
        coef_math(gsb11, 1, a1p[1], c1b[1], 2, 1, 3, th2[1], 5)
        for b in range(4):
            phaseB_block(1, b)

        for m in range(NMT):
            conv2_mega(0, m)
        gsb20 = emit_stats(bnst2[0], "20")
        for m in range(NMT):
            conv2_mega(1, m)
        gsb21 = emit_stats(bnst2[1], "21")
        coef_math(gsb20, 0, a2p[0], c2b[0], 7, 6, 8)
        for b in range(8):
            phaseD_block(0, b)
        coef_math(gsb21, 1, a2p[1], c2b[1], 7, 6, 8)
        for b in range(8):
            phaseD_block(1, b)

    nc.compile()
    return nc


def _get_nc():
    if "nc" not in _nc_cache:
        _nc_cache["nc"] = _build()
    return _nc_cache["nc"]


def _prep_inputs(inputs):
    f8np = mybir.dt.np(F8)
    x = np.ascontiguousarray(np.asarray(inputs["x"], np.float32))
    w3 = np.asarray(inputs["w3x3"], np.float32)
    wr = np.asarray(inputs["wres"], np.float32)
    s1 = np.abs(w3).mean(axis=(1, 2, 3))
    # conv2 activations are +-0.5 on device -> fold 2x into its scale
    s2 = 2.0 * np.abs(wr).mean(axis=(1, 2, 3))
    w1h = (np.sign(w3).reshape(2, P, 2, P, 3, 3).transpose(3, 0, 4, 5, 2, 1)
           .reshape(P, 18, 2, P)).astype(f8np)
    w2h = (np.sign(wr)[:, :, 0, 0].reshape(2, P, 2, P).transpose(3, 0, 2, 1)
           .reshape(P, 2, 2, P)).astype(f8np)

    def col(v):
        return np.asarray(v, np.float32).reshape(2, P).T

    g1 = np.asarray(inputs["bn1_g"], np.float32)
    be1 = np.asarray(inputs["bn1_b"], np.float32)
    g2 = np.asarray(inputs["bn2_g"], np.float32)
    be2 = np.asarray(inputs["bn2_b"], np.float32)
    b1_1, b1_2, b1_3 = (np.asarray(inputs[k], np.float32)
                        for k in ("b1_1", "b1_2", "b1_3"))
    b2_1, b2_2, b2_3 = (np.asarray(inputs[k], np.float32)
                        for k in ("b2_1", "b2_2", "b2_3"))
    pa1 = np.asarray(inputs["prelu1_a"], np.float32)
    pa2 = np.asarray(inputs["prelu2_a"], np.float32)
    # sign2 threshold offset: sign(Prelu(z)+b') == sign(z + t2), b'=b1_3+b2_1
    bp = b1_3 + b2_1
    t2 = np.where(bp >= 0, bp / pa1, bp).astype(np.float32)
    cols = [b1_1, g1 * s1, s1 * s1, be1 + b1_2, pa1, t2,
            g2 * s2, s2 * s2, be2 + b1_3 + b2_2, pa2, b2_3]
    csth = np.stack([col(v) for v in cols], axis=2).astype(np.float32)
    csth = np.ascontiguousarray(csth)

    in_maps = []
    for c in range(NCORES):
        in_maps.append({
            "x": np.ascontiguousarray(x[c * IMGS:(c + 1) * IMGS]),
            "w1": w1h, "w2": w2h, "consts": csth,
        })
    return in_maps


def _run(in_maps, trace=False):
    nc = _get_nc()
    return bass_utils.run_bass_kernel_spmd(
        nc, in_maps, core_ids=list(range(NCORES)), trace=trace)


def kernel(**inputs):
    in_maps = _prep_inputs(inputs)
    res = _run(in_maps)
    out = np.concatenate([res.results[c]["out"] for c in range(NCORES)], axis=0)
    return out.astype(np.float32)
